# revision 26
# baseline (speedup 1.0000x reference)
"""Trainium2 Bass kernel for nn_ClusterMemory (scatter_memory).

Contract: kernel(**inputs) takes FULL unsharded inputs (inputs1 [256,256] f32,
inputs2 [256,256] f32, features [100000,256] f32, targets [256] int) and
returns the FULL output tuple (loss_c, loss_s, new_features[100000,256] f32),
matching reference.reference().

Strategy (per sharding hint): shard the memory bank (features) along N across
the 8 NeuronCores (12500 rows each).  Host also passes a bf16 copy of each
shard (padded to a multiple of 16 rows).  Each core:
  - streams the f32 shard through SBUF once (contiguous 8KB-per-partition
    DMA layout) and writes it back as its shard of new_features
    (bit-exact pass-through for untouched rows),
  - xbar-DMA-transpose-loads the bf16 shard DRAM->SBUF to get F^T with the
    contraction dim on partitions (the transpose IS the load -- no PE
    transposes, no casts, no PSUM->SBUF copies),
  - computes OUT = x1n @ F_s^T and REG = x2n @ F_s^T via bf16 matmuls
    (f32 PSUM accumulate),
  - accumulates per-batch-row partial softmax sums:
        l_out[b] = sum_n exp(20*out - 20)        (ACT fused exp+row-sum)
        l_reg[b] = sum_n exp(20*reg - 20)
        t1[b]    = sum_n exp(20*reg - 20)*out    (DVE mult + DVE row-sum)
    (all rows are unit-norm, so |logit| <= 1/TEMP = 20: a FIXED shift of 20
     is exact stabilization -- no cross-shard max reduction needed; the n
     permutation from the f32 DMA layout is irrelevant because all
     reductions over n are permutation-invariant).
Host combines the 8x[256] partial sums into the two scalar losses, computes
the <=256 segment-softmax-updated rows (local to the shard that owns each
target index; B=256 so this is tiny), and scatters them into the gathered
output.
"""

import ml_dtypes
import numpy as np

import concourse.bacc as bacc
import concourse.bass as bass
import concourse.tile as tile
from concourse import mybir
from concourse.bass_utils import run_bass_kernel_spmd

# Problem constants (hardcoded per harness contract)
N_CORES = 8
N_TOTAL = 100000
NS = N_TOTAL // N_CORES  # 12500 rows per shard
NSB = 12512  # bf16 shard rows padded to a multiple of 16 (xbar tile rows)
D = 256
B = 256
P = 128
TEMP = 0.05
INV_TEMP = 1.0 / TEMP  # 20.0
SHIFT = 20.0  # |dot| <= 1 (unit rows) -> |logit| <= 20
MOMENTUM = 0.2
SMOOTH = 0.1
TAU_W = 0.09

CH = 1024  # feature rows per chunk (main chunks)

# knobs for test.py
TRACE = False
LAST_RESULTS = None

_PROG = None


def _build_program():
    f32 = mybir.dt.float32
    bf16 = mybir.dt.bfloat16
    nc = bacc.Bacc(
        "TRN2", target_bir_lowering=False, debug=False, enable_asserts=False
    )

    feat = nc.dram_tensor("feat", [NS, D], f32, kind="ExternalInput").ap()
    featb = nc.dram_tensor("featb", [NSB, D], bf16, kind="ExternalInput").ap()
    x1t = nc.dram_tensor("x1t", [D, B], f32, kind="ExternalInput").ap()
    x2t = nc.dram_tensor("x2t", [D, B], f32, kind="ExternalInput").ap()
    newfeat = nc.dram_tensor("newfeat", [NS, D], f32, kind="ExternalOutput").ap()
    stats = nc.dram_tensor("stats", [P, 6], f32, kind="ExternalOutput").ap()

    n_main = NS // CH  # 12 main chunks
    rag_rows = NS - n_main * CH  # 212 ragged rows
    rag_pad = NSB - n_main * CH  # 224 (multiple of 16)
    n_sub = n_main + (1 if rag_rows else 0)

    Exp = mybir.ActivationFunctionType.Exp

    with tile.TileContext(nc) as tc:
        with (
            tc.tile_pool(name="const", bufs=1) as const_pool,
            tc.tile_pool(name="feat", bufs=4) as feat_pool,
            tc.tile_pool(name="ft", bufs=4) as ft_pool,
            tc.tile_pool(name="ereg", bufs=3) as ereg_pool,
            tc.tile_pool(name="scr", bufs=3) as scr_pool,
            tc.tile_pool(name="stat", bufs=1) as stat_pool,
            tc.tile_pool(name="ps", bufs=4, space="PSUM") as psum_pool,
        ):
            neg_shift = const_pool.tile([P, 1], f32)
            nc.gpsimd.memset(neg_shift[:], -SHIFT)

            x1sb_f = const_pool.tile([P, 2, B], f32)
            nc.sync.dma_start(x1sb_f[:], x1t.rearrange("(k p) b -> p k b", p=P))
            x2sb_f = const_pool.tile([P, 2, B], f32)
            nc.sync.dma_start(x2sb_f[:], x2t.rearrange("(k p) b -> p k b", p=P))
            x1sb = const_pool.tile([P, 2, B], bf16)
            nc.vector.tensor_copy(x1sb[:], x1sb_f[:])
            x2sb = const_pool.tile([P, 2, B], bf16)
            nc.vector.tensor_copy(x2sb[:], x2sb_f[:])

            # per-(quantity, b-half) partial-sum columns, one col per chunk
            cols = {}
            for q in range(3):
                for h in range(2):
                    cols[(q, h)] = stat_pool.tile(
                        [P, n_sub], f32, tag=f"cols{q}{h}", name=f"cols{q}{h}"
                    )

            def compute_sums(po, pr, width, ci):
                """exp+accumulate over psum OUT/REG tiles [128, width]."""
                for h in range(2):
                    scr = scr_pool.tile([P, CH], bf16, tag="scr")
                    nc.scalar.activation(
                        scr[:, 0:width],
                        po[h][:, 0:width],
                        Exp,
                        bias=neg_shift[:],
                        scale=INV_TEMP,
                        accum_out=cols[(0, h)][:, ci : ci + 1],
                    )
                    er = ereg_pool.tile([P, CH], bf16, tag="ereg")
                    nc.scalar.activation(
                        er[:, 0:width],
                        pr[h][:, 0:width],
                        Exp,
                        bias=neg_shift[:],
                        scale=INV_TEMP,
                        accum_out=cols[(1, h)][:, ci : ci + 1],
                    )
                    scr2 = scr_pool.tile([P, CH], bf16, tag="scr2")
                    nc.vector.tensor_mul(
                        scr2[:, 0:width], er[:, 0:width], po[h][:, 0:width]
                    )
                    nc.vector.reduce_sum(
                        out=cols[(2, h)][:, ci : ci + 1],
                        in_=scr2[:, 0:width],
                        axis=mybir.AxisListType.X,
                    )

            def emit_chunk(row0, nrows, nrows_pad, ci):
                # f32 pass-through: load + store
                fs = feat_pool.tile([P, 8, D], f32, tag="fs")
                full = (nrows // P) * P
                jr = nrows // P
                rem = nrows - full
                if full == nrows and nrows == CH:
                    nc.gpsimd.dma_start(
                        fs[:],
                        feat[row0 : row0 + CH, :].rearrange(
                            "(p j) d -> p j d", p=P
                        ),
                    )
                    nc.gpsimd.dma_start(
                        newfeat[row0 : row0 + CH, :].rearrange(
                            "(p j) d -> p j d", p=P
                        ),
                        fs[:],
                    )
                else:
                    if full:
                        nc.gpsimd.dma_start(
                            fs[:, 0:jr, :],
                            feat[row0 : row0 + full, :].rearrange(
                                "(j p) d -> p j d", p=P
                            ),
                        )
                        nc.gpsimd.dma_start(
                            newfeat[row0 : row0 + full, :].rearrange(
                                "(j p) d -> p j d", p=P
                            ),
                            fs[:, 0:jr, :],
                        )
                    if rem:
                        nc.gpsimd.dma_start(
                            fs[:rem, jr, :],
                            feat[row0 + full : row0 + nrows, :],
                        )
                        nc.gpsimd.dma_start(
                            newfeat[row0 + full : row0 + nrows, :],
                            fs[:rem, jr, :],
                        )
                # xbar transpose-load of the bf16 shard: DRAM -> SBUF
                # ftb[d, k, r] = featb[row0 + r, k*128 + d]
                ftb = ft_pool.tile([P, 2, CH], bf16, tag="ftb")
                nc.sync.dma_start(
                    ftb[:, :, 0:nrows_pad],
                    featb[row0 : row0 + nrows_pad, :],
                    transpose=True,
                )
                # OUT/REG = x^T.T @ F^T, accumulated over the two d-blocks
                po = []
                pr = []
                for xsb, lst in ((x2sb, pr), (x1sb, po)):
                    for h in range(2):
                        pt = psum_pool.tile([P, CH], f32, tag="ps")
                        for k in range(2):
                            for n0 in range(0, nrows_pad, 512):
                                nw = min(512, nrows_pad - n0)
                                nc.tensor.matmul(
                                    pt[:, n0 : n0 + nw],
                                    xsb[:, k, h * P : (h + 1) * P],
                                    ftb[:, k, n0 : n0 + nw],
                                    start=(k == 0),
                                    stop=(k == 1),
                                )
                        lst.append(pt)
                compute_sums(po, pr, nrows, ci)

            for ci in range(n_main):
                emit_chunk(ci * CH, CH, CH, ci)
            if rag_rows:
                emit_chunk(n_main * CH, rag_rows, rag_pad, n_main)

            stat_sb = stat_pool.tile([P, 6], f32)
            for q in range(3):
                for h in range(2):
                    nc.vector.reduce_sum(
                        out=stat_sb[:, q * 2 + h : q * 2 + h + 1],
                        in_=cols[(q, h)][:],
                        axis=mybir.AxisListType.X,
                    )
            nc.sync.dma_start(stats[:], stat_sb[:])

    nc.compile()
    return nc


def _get_program():
    global _PROG
    if _PROG is None:
        _PROG = _build_program()
    return _PROG


def _l2norm_rows(x):
    n = np.sqrt((x.astype(np.float64) ** 2).sum(axis=1, keepdims=True))
    return (x / np.maximum(n, 1e-12)).astype(np.float32)


def kernel(inputs1, inputs2, features, targets):
    global LAST_RESULTS
    inputs1 = np.ascontiguousarray(np.asarray(inputs1, dtype=np.float32))
    inputs2 = np.ascontiguousarray(np.asarray(inputs2, dtype=np.float32))
    features = np.ascontiguousarray(np.asarray(features, dtype=np.float32))
    targets_np = np.asarray(targets)
    tgt = targets_np.astype(np.int64)

    x1 = _l2norm_rows(inputs1)
    x2 = _l2norm_rows(inputs2)
    x1t = np.ascontiguousarray(x1.T)
    x2t = np.ascontiguousarray(x2.T)

    featb = np.zeros((N_CORES, NSB, D), dtype=ml_dtypes.bfloat16)
    featb[:, :NS, :] = features.reshape(N_CORES, NS, D)

    nc = _get_program()
    in_maps = [
        {
            "feat": features[c * NS : (c + 1) * NS],
            "featb": featb[c],
            "x1t": x1t,
            "x2t": x2t,
        }
        for c in range(N_CORES)
    ]
    res = run_bass_kernel_spmd(
        nc, in_maps, core_ids=list(range(N_CORES)), trace=TRACE
    )
    LAST_RESULTS = res

    # ---- combine per-shard softmax partial sums (host, tiny) ----
    # stats cols: 0,1 = l_out (b 0:128, 128:256); 2,3 = l_reg; 4,5 = t1
    st = np.stack([res.results[c]["stats"] for c in range(N_CORES)], axis=0)
    st = st.astype(np.float64)
    l_out = st[:, :, 0:2].sum(axis=0).T.reshape(B)  # [B]
    l_reg = st[:, :, 2:4].sum(axis=0).T.reshape(B)
    t1 = st[:, :, 4:6].sum(axis=0).T.reshape(B)

    lse_out = SHIFT + np.log(l_out)  # log sum_n exp(o[b,n]), o = 20*dot
    s1 = INV_TEMP * t1 / l_reg  # sum_n softmax(reg)[b,n] * o[b,n]

    x1_64 = x1.astype(np.float64)
    frows = features[tgt].astype(np.float64)
    d = (x1_64 * frows).sum(axis=1)  # [B] sim to own centroid
    o_t = INV_TEMP * d

    loss_c = np.mean(lse_out - o_t)
    loss_s = np.mean(
        (1.0 - SMOOTH) * (lse_out - s1) + SMOOTH * (lse_out - o_t)
    )

    # ---- segment-softmax memory update rows (host, <=256 rows) ----
    logits_w = -d / TAU_W
    uniq, inv = np.unique(tgt, return_inverse=True)
    nu = len(uniq)
    m_u = np.full(nu, -np.inf)
    np.maximum.at(m_u, inv, logits_w)
    e = np.exp(logits_w - m_u[inv])
    den_u = np.zeros(nu)
    np.add.at(den_u, inv, e)
    wgt = e / den_u[inv]
    wmean_u = np.zeros((nu, D))
    np.add.at(wmean_u, inv, wgt[:, None] * x1_64)
    upd = MOMENTUM * features[uniq].astype(np.float64) + (1.0 - MOMENTUM) * wmean_u
    un = np.sqrt((upd**2).sum(axis=1, keepdims=True))
    upd = (upd / np.maximum(un, 1e-12)).astype(np.float32)

    # ---- gather shards + scatter updated rows ----
    new_features = np.concatenate(
        [res.results[c]["newfeat"] for c in range(N_CORES)], axis=0
    )
    new_features[uniq] = upd

    return (
        np.float32(loss_c),
        np.float32(loss_s),
        new_features,
    )


# revision 27
# speedup vs baseline: 1.4625x; 1.4625x over previous
"""Trainium2 Bass kernel for nn_ClusterMemory (scatter_memory).

Contract: kernel(**inputs) takes FULL unsharded inputs (inputs1 [256,256] f32,
inputs2 [256,256] f32, features [100000,256] f32, targets [256] int) and
returns the FULL output tuple (loss_c, loss_s, new_features[100000,256] f32),
matching reference.reference().

Strategy (per sharding hint): shard the memory bank (features) along N across
the 8 NeuronCores (12500 rows each).  Each core:
  - streams its feature shard through SBUF once,
  - PE-transposes 128x128 blocks to get F^T (contraction dim on partitions),
  - computes OUT = x1n @ F_s^T and REG = x2n @ F_s^T via f32r matmuls,
  - accumulates per-row-of-batch partial softmax sums:
        l_out[b] = sum_n exp(20*out - 20)        (ACT fused exp+row-sum)
        l_reg[b] = sum_n exp(20*reg - 20)
        t1[b]    = sum_n exp(20*reg - 20)*out    (DVE fused mul+row-sum)
    (all rows are unit-norm, so |logit| <= 1/TEMP = 20: a FIXED shift of 20
     is exact stabilization -- no cross-shard max reduction needed),
  - writes its shard of new_features as a pass-through copy from the same
    SBUF tiles (bit-exact for untouched rows).
Host combines the 8x[256] partial sums into the two scalar losses, computes
the <=256 segment-softmax-updated rows (local to the shard that owns each
target index; B=256 so this is tiny), and scatters them into the gathered
output.
"""

import numpy as np

import concourse.bacc as bacc
import concourse.bass as bass
import concourse.tile as tile
from concourse import mybir
from concourse.bass_utils import run_bass_kernel_spmd
from concourse.masks import make_identity

# Problem constants (hardcoded per harness contract)
N_CORES = 8
N_TOTAL = 100000
NS = N_TOTAL // N_CORES  # 12500 rows per shard
D = 256
B = 256
P = 128
TEMP = 0.05
INV_TEMP = 1.0 / TEMP  # 20.0
SHIFT = 20.0  # |dot| <= 1 (unit rows) -> |logit| <= 20
MOMENTUM = 0.2
SMOOTH = 0.1
TAU_W = 0.09

CH = 1024  # feature rows per DMA chunk
SUB = 512  # columns (n) per compute subchunk

# knobs for test.py
TRACE = False
LAST_RESULTS = None

_PROG = None


def _build_program():
    f32 = mybir.dt.float32
    f32r = mybir.dt.float32r
    nc = bacc.Bacc(
        "TRN2", target_bir_lowering=False, debug=False, enable_asserts=False
    )

    feat = nc.dram_tensor("feat", [NS, D], f32, kind="ExternalInput").ap()
    x1t = nc.dram_tensor("x1t", [D, B], f32, kind="ExternalInput").ap()
    x2t = nc.dram_tensor("x2t", [D, B], f32, kind="ExternalInput").ap()
    newfeat = nc.dram_tensor("newfeat", [NS, D], f32, kind="ExternalOutput").ap()
    stats = nc.dram_tensor("stats", [P, 6], f32, kind="ExternalOutput").ap()

    # chunk schedule: 12 x 1024 rows + 1 x 212 rows = 12500
    chunks = [(i * CH, CH) for i in range(NS // CH)]
    if NS % CH:
        chunks.append((NS - NS % CH, NS % CH))
    n_sub = sum((nrows + SUB - 1) // SUB for _, nrows in chunks)

    Exp = mybir.ActivationFunctionType.Exp

    with tile.TileContext(nc) as tc:
        with (
            tc.tile_pool(name="const", bufs=1) as const_pool,
            tc.tile_pool(name="feat", bufs=4) as feat_pool,
            tc.tile_pool(name="ft", bufs=4) as ft_pool,
            tc.tile_pool(name="ereg", bufs=3) as ereg_pool,
            tc.tile_pool(name="scr", bufs=3) as scr_pool,
            tc.tile_pool(name="stat", bufs=1) as stat_pool,
            tc.tile_pool(name="ps", bufs=8, space="PSUM") as psum_pool,
        ):
            ident = const_pool.tile([P, P], f32)
            make_identity(nc, ident[:])
            neg_shift = const_pool.tile([P, 1], f32)
            nc.gpsimd.memset(neg_shift[:], -SHIFT)

            x1sb_f = const_pool.tile([P, 2, B], f32)
            nc.sync.dma_start(x1sb_f[:], x1t.rearrange("(k p) b -> p k b", p=P))
            x2sb_f = const_pool.tile([P, 2, B], f32)
            nc.sync.dma_start(x2sb_f[:], x2t.rearrange("(k p) b -> p k b", p=P))
            # round to f32r once (verifier: f32r matmul operands must be
            # produced as f32r)
            x1sb = const_pool.tile([P, 2, B], f32r)
            nc.vector.tensor_copy(x1sb[:], x1sb_f[:])
            x2sb = const_pool.tile([P, 2, B], f32r)
            nc.vector.tensor_copy(x2sb[:], x2sb_f[:])

            # per-(quantity, b-half) partial-sum columns, one col per subchunk
            cols = {}
            for q in range(3):
                for h in range(2):
                    cols[(q, h)] = stat_pool.tile(
                        [P, n_sub], f32, tag=f"cols{q}{h}", name=f"cols{q}{h}"
                    )

            si = 0
            for row0, nrows in chunks:
                fs = feat_pool.tile([P, CH // P, D], f32, tag="fs")
                full = (nrows // P) * P
                rem = nrows - full
                jr = nrows // P
                if nrows == CH:
                    # row = row0 + p*8 + j: 8KB contiguous per partition
                    nc.sync.dma_start(
                        fs[:],
                        feat[row0 : row0 + CH, :].rearrange(
                            "(p j) d -> p j d", p=P
                        ),
                    )
                    nc.sync.dma_start(
                        newfeat[row0 : row0 + CH, :].rearrange(
                            "(p j) d -> p j d", p=P
                        ),
                        fs[:],
                    )
                else:
                    if full:
                        nc.sync.dma_start(
                            fs[:, 0:jr, :],
                            feat[row0 : row0 + full, :].rearrange(
                                "(j p) d -> p j d", p=P
                            ),
                        )
                        nc.sync.dma_start(
                            newfeat[row0 : row0 + full, :].rearrange(
                                "(j p) d -> p j d", p=P
                            ),
                            fs[:, 0:jr, :],
                        )
                    if rem:
                        nc.sync.dma_start(
                            fs[:rem, jr, :],
                            feat[row0 + full : row0 + nrows, :],
                        )
                        nc.sync.dma_start(
                            newfeat[row0 + full : row0 + nrows, :],
                            fs[:rem, jr, :],
                        )

                for s0 in range(0, nrows, SUB):
                    w = min(SUB, nrows - s0)
                    jbase = s0 // P
                    # transpose F blocks: fs[n_p, j, d] -> pt[k][d_p, n]
                    ftt = ft_pool.tile([P, 2, SUB], f32r, tag="ft")
                    pts = []
                    for k in range(2):
                        pt = psum_pool.tile([P, SUB], f32, tag="ps")
                        off = 0
                        j = jbase
                        while off < w:
                            bw = min(P, w - off)
                            nc.tensor.transpose(
                                pt[:, off : off + bw],
                                fs[:bw, j, k * P : (k + 1) * P],
                                ident[:bw, 0:bw],
                            )
                            off += bw
                            j += 1
                        pts.append(pt)
                    for k in range(2):
                        nc.vector.tensor_copy(ftt[:, k, 0:w], pts[k][:, 0:w])

                    # OUT/REG = x^T.T @ F^T, accumulated over the two d-blocks
                    po = []
                    pr = []
                    for xsb, lst in ((x2sb, pr), (x1sb, po)):
                        for h in range(2):
                            pt = psum_pool.tile([P, SUB], f32, tag="ps")
                            for k in range(2):
                                nc.tensor.matmul(
                                    pt[:, 0:w],
                                    xsb[:, k, h * P : (h + 1) * P],
                                    ftt[:, k, 0:w],
                                    start=(k == 0),
                                    stop=(k == 1),
                                )
                            lst.append(pt)

                    for h in range(2):
                        er = ereg_pool.tile([P, SUB], f32, tag="ereg")
                        nc.scalar.activation(
                            er[:, 0:w],
                            pr[h][:, 0:w],
                            Exp,
                            bias=neg_shift[:],
                            scale=INV_TEMP,
                            accum_out=cols[(1, h)][:, si : si + 1],
                        )
                        scr = scr_pool.tile([P, SUB], f32, tag="scr")
                        nc.scalar.activation(
                            scr[:, 0:w],
                            po[h][:, 0:w],
                            Exp,
                            bias=neg_shift[:],
                            scale=INV_TEMP,
                            accum_out=cols[(0, h)][:, si : si + 1],
                        )
                        # t1 partial: sum_n e_reg * out_raw
                        # (tensor_tensor_reduce crashes on this HW/runtime --
                        #  use plain mult + reduce on DVE instead)
                        scr2 = scr_pool.tile([P, SUB], f32, tag="scr")
                        nc.vector.tensor_mul(
                            scr2[:, 0:w], er[:, 0:w], po[h][:, 0:w]
                        )
                        nc.vector.reduce_sum(
                            out=cols[(2, h)][:, si : si + 1],
                            in_=scr2[:, 0:w],
                            axis=mybir.AxisListType.X,
                        )
                    si += 1

            stat_sb = stat_pool.tile([P, 6], f32)
            for q in range(3):
                for h in range(2):
                    nc.vector.reduce_sum(
                        out=stat_sb[:, q * 2 + h : q * 2 + h + 1],
                        in_=cols[(q, h)][:],
                        axis=mybir.AxisListType.X,
                    )
            nc.sync.dma_start(stats[:], stat_sb[:])

    nc.compile()
    return nc


def _get_program():
    global _PROG
    if _PROG is None:
        _PROG = _build_program()
    return _PROG


def _l2norm_rows(x):
    n = np.sqrt((x.astype(np.float64) ** 2).sum(axis=1, keepdims=True))
    return (x / np.maximum(n, 1e-12)).astype(np.float32)


def kernel(inputs1, inputs2, features, targets):
    global LAST_RESULTS
    inputs1 = np.ascontiguousarray(np.asarray(inputs1, dtype=np.float32))
    inputs2 = np.ascontiguousarray(np.asarray(inputs2, dtype=np.float32))
    features = np.ascontiguousarray(np.asarray(features, dtype=np.float32))
    targets_np = np.asarray(targets)
    tgt = targets_np.astype(np.int64)

    x1 = _l2norm_rows(inputs1)
    x2 = _l2norm_rows(inputs2)
    x1t = np.ascontiguousarray(x1.T)
    x2t = np.ascontiguousarray(x2.T)

    nc = _get_program()
    in_maps = [
        {
            "feat": features[c * NS : (c + 1) * NS],
            "x1t": x1t,
            "x2t": x2t,
        }
        for c in range(N_CORES)
    ]
    res = run_bass_kernel_spmd(
        nc, in_maps, core_ids=list(range(N_CORES)), trace=TRACE
    )
    LAST_RESULTS = res

    # ---- combine per-shard softmax partial sums (host, tiny) ----
    # stats cols: 0,1 = l_out (b 0:128, 128:256); 2,3 = l_reg; 4,5 = t1
    st = np.stack([res.results[c]["stats"] for c in range(N_CORES)], axis=0)
    st = st.astype(np.float64)
    l_out = st[:, :, 0:2].sum(axis=0).T.reshape(B)  # [B]
    l_reg = st[:, :, 2:4].sum(axis=0).T.reshape(B)
    t1 = st[:, :, 4:6].sum(axis=0).T.reshape(B)

    lse_out = SHIFT + np.log(l_out)  # log sum_n exp(o[b,n]), o = 20*dot
    s1 = INV_TEMP * t1 / l_reg  # sum_n softmax(reg)[b,n] * o[b,n]

    x1_64 = x1.astype(np.float64)
    frows = features[tgt].astype(np.float64)
    d = (x1_64 * frows).sum(axis=1)  # [B] sim to own centroid
    o_t = INV_TEMP * d

    loss_c = np.mean(lse_out - o_t)
    loss_s = np.mean(
        (1.0 - SMOOTH) * (lse_out - s1) + SMOOTH * (lse_out - o_t)
    )

    # ---- segment-softmax memory update rows (host, <=256 rows) ----
    logits_w = -d / TAU_W
    uniq, inv = np.unique(tgt, return_inverse=True)
    nu = len(uniq)
    m_u = np.full(nu, -np.inf)
    np.maximum.at(m_u, inv, logits_w)
    e = np.exp(logits_w - m_u[inv])
    den_u = np.zeros(nu)
    np.add.at(den_u, inv, e)
    wgt = e / den_u[inv]
    wmean_u = np.zeros((nu, D))
    np.add.at(wmean_u, inv, wgt[:, None] * x1_64)
    upd = MOMENTUM * features[uniq].astype(np.float64) + (1.0 - MOMENTUM) * wmean_u
    un = np.sqrt((upd**2).sum(axis=1, keepdims=True))
    upd = (upd / np.maximum(un, 1e-12)).astype(np.float32)

    # ---- gather shards + scatter updated rows ----
    new_features = np.concatenate(
        [res.results[c]["newfeat"] for c in range(N_CORES)], axis=0
    )
    new_features[uniq] = upd

    return (
        np.float32(loss_c),
        np.float32(loss_s),
        new_features,
    )


# revision 28
# speedup vs baseline: 1.5912x; 1.0880x over previous
"""Trainium2 Bass kernel for nn_ClusterMemory (scatter_memory).

Contract: kernel(**inputs) takes FULL unsharded inputs (inputs1 [256,256] f32,
inputs2 [256,256] f32, features [100000,256] f32, targets [256] int) and
returns the FULL output tuple (loss_c, loss_s, new_features[100000,256] f32),
matching reference.reference().

Strategy (per sharding hint): shard the memory bank (features) along N across
the 8 NeuronCores (12500 rows each).  Each core:
  - streams its feature shard through SBUF once,
  - PE-transposes 128x128 blocks to get F^T (contraction dim on partitions),
  - computes OUT = x1n @ F_s^T and REG = x2n @ F_s^T via f32r matmuls,
  - accumulates per-row-of-batch partial softmax sums:
        l_out[b] = sum_n exp(20*out - 20)        (ACT fused exp+row-sum)
        l_reg[b] = sum_n exp(20*reg - 20)
        t1[b]    = sum_n exp(20*reg - 20)*out    (DVE fused mul+row-sum)
    (all rows are unit-norm, so |logit| <= 1/TEMP = 20: a FIXED shift of 20
     is exact stabilization -- no cross-shard max reduction needed),
  - writes its shard of new_features as a pass-through copy from the same
    SBUF tiles (bit-exact for untouched rows).
Host combines the 8x[256] partial sums into the two scalar losses, computes
the <=256 segment-softmax-updated rows (local to the shard that owns each
target index; B=256 so this is tiny), and scatters them into the gathered
output.
"""

import numpy as np

import concourse.bacc as bacc
import concourse.bass as bass
import concourse.tile as tile
from concourse import mybir
from concourse.bass_utils import run_bass_kernel_spmd
from concourse.masks import make_identity

# Problem constants (hardcoded per harness contract)
N_CORES = 8
N_TOTAL = 100000
NS = N_TOTAL // N_CORES  # 12500 rows per shard
D = 256
B = 256
P = 128
TEMP = 0.05
INV_TEMP = 1.0 / TEMP  # 20.0
SHIFT = 20.0  # |dot| <= 1 (unit rows) -> |logit| <= 20
MOMENTUM = 0.2
SMOOTH = 0.1
TAU_W = 0.09

CH = 1024  # feature rows per DMA chunk
SUB = 512  # columns (n) per compute subchunk

# knobs for test.py
TRACE = False
LAST_RESULTS = None

_PROG = None


def _build_program():
    f32 = mybir.dt.float32
    f32r = mybir.dt.float32r
    nc = bacc.Bacc(
        "TRN2", target_bir_lowering=False, debug=False, enable_asserts=False
    )

    feat = nc.dram_tensor("feat", [NS, D], f32, kind="ExternalInput").ap()
    x1t = nc.dram_tensor("x1t", [D, B], f32, kind="ExternalInput").ap()
    x2t = nc.dram_tensor("x2t", [D, B], f32, kind="ExternalInput").ap()
    newfeat = nc.dram_tensor("newfeat", [NS, D], f32, kind="ExternalOutput").ap()
    stats = nc.dram_tensor("stats", [P, 6], f32, kind="ExternalOutput").ap()

    # chunk schedule: 12 x 1024 rows + 1 x 212 rows = 12500
    chunks = [(i * CH, CH) for i in range(NS // CH)]
    if NS % CH:
        chunks.append((NS - NS % CH, NS % CH))
    n_sub = sum((nrows + SUB - 1) // SUB for _, nrows in chunks)

    Exp = mybir.ActivationFunctionType.Exp

    with tile.TileContext(nc) as tc:
        with (
            tc.tile_pool(name="const", bufs=1) as const_pool,
            tc.tile_pool(name="feat", bufs=4) as feat_pool,
            tc.tile_pool(name="ft", bufs=4) as ft_pool,
            tc.tile_pool(name="ereg", bufs=3) as ereg_pool,
            tc.tile_pool(name="scr", bufs=3) as scr_pool,
            tc.tile_pool(name="stat", bufs=1) as stat_pool,
            tc.tile_pool(name="ps", bufs=8, space="PSUM") as psum_pool,
        ):
            ident = const_pool.tile([P, P], f32)
            make_identity(nc, ident[:])
            neg_shift = const_pool.tile([P, 1], f32)
            nc.gpsimd.memset(neg_shift[:], -SHIFT)

            x1sb_f = const_pool.tile([P, 2, B], f32)
            nc.sync.dma_start(x1sb_f[:], x1t.rearrange("(k p) b -> p k b", p=P))
            x2sb_f = const_pool.tile([P, 2, B], f32)
            nc.sync.dma_start(x2sb_f[:], x2t.rearrange("(k p) b -> p k b", p=P))
            # round to f32r once (verifier: f32r matmul operands must be
            # produced as f32r)
            x1sb = const_pool.tile([P, 2, B], f32r)
            nc.vector.tensor_copy(x1sb[:], x1sb_f[:])
            x2sb = const_pool.tile([P, 2, B], f32r)
            nc.vector.tensor_copy(x2sb[:], x2sb_f[:])

            # per-(quantity, b-half) partial-sum columns, one col per subchunk
            cols = {}
            for q in range(3):
                for h in range(2):
                    cols[(q, h)] = stat_pool.tile(
                        [P, n_sub], f32, tag=f"cols{q}{h}", name=f"cols{q}{h}"
                    )

            si = 0
            for row0, nrows in chunks:
                fs = feat_pool.tile([P, CH // P, D], f32, tag="fs")
                full = (nrows // P) * P
                rem = nrows - full
                jr = nrows // P
                if nrows == CH:
                    # row = row0 + p*8 + j: 8KB contiguous per partition
                    nc.sync.dma_start(
                        fs[:],
                        feat[row0 : row0 + CH, :].rearrange(
                            "(p j) d -> p j d", p=P
                        ),
                    )
                    nc.sync.dma_start(
                        newfeat[row0 : row0 + CH, :].rearrange(
                            "(p j) d -> p j d", p=P
                        ),
                        fs[:],
                    )
                else:
                    if full:
                        nc.sync.dma_start(
                            fs[:, 0:jr, :],
                            feat[row0 : row0 + full, :].rearrange(
                                "(j p) d -> p j d", p=P
                            ),
                        )
                        nc.sync.dma_start(
                            newfeat[row0 : row0 + full, :].rearrange(
                                "(j p) d -> p j d", p=P
                            ),
                            fs[:, 0:jr, :],
                        )
                    if rem:
                        nc.sync.dma_start(
                            fs[:rem, jr, :],
                            feat[row0 + full : row0 + nrows, :],
                        )
                        nc.sync.dma_start(
                            newfeat[row0 + full : row0 + nrows, :],
                            fs[:rem, jr, :],
                        )

                for s0 in range(0, nrows, SUB):
                    w = min(SUB, nrows - s0)
                    jbase = s0 // P
                    # transpose F blocks: fs[n_p, j, d] -> pt[k][d_p, n]
                    ftt = ft_pool.tile([P, 2, SUB], f32r, tag="ft")
                    pts = []
                    for k in range(2):
                        pt = psum_pool.tile([P, SUB], f32, tag="ps")
                        off = 0
                        j = jbase
                        while off < w:
                            bw = min(P, w - off)
                            nc.tensor.transpose(
                                pt[:, off : off + bw],
                                fs[:bw, j, k * P : (k + 1) * P],
                                ident[:bw, 0:bw],
                            )
                            off += bw
                            j += 1
                        pts.append(pt)
                    for k in range(2):
                        nc.vector.tensor_copy(ftt[:, k, 0:w], pts[k][:, 0:w])

                    # OUT/REG = x^T.T @ F^T, accumulated over the two d-blocks
                    po = []
                    pr = []
                    for xsb, lst in ((x2sb, pr), (x1sb, po)):
                        for h in range(2):
                            pt = psum_pool.tile([P, SUB], f32, tag="ps")
                            for k in range(2):
                                nc.tensor.matmul(
                                    pt[:, 0:w],
                                    xsb[:, k, h * P : (h + 1) * P],
                                    ftt[:, k, 0:w],
                                    start=(k == 0),
                                    stop=(k == 1),
                                )
                            lst.append(pt)

                    for h in range(2):
                        er = ereg_pool.tile([P, SUB], f32, tag="ereg")
                        nc.scalar.activation(
                            er[:, 0:w],
                            pr[h][:, 0:w],
                            Exp,
                            bias=neg_shift[:],
                            scale=INV_TEMP,
                            accum_out=cols[(1, h)][:, si : si + 1],
                        )
                        scr = scr_pool.tile([P, SUB], f32, tag="scr")
                        nc.scalar.activation(
                            scr[:, 0:w],
                            po[h][:, 0:w],
                            Exp,
                            bias=neg_shift[:],
                            scale=INV_TEMP,
                            accum_out=cols[(0, h)][:, si : si + 1],
                        )
                        # t1 partial: sum_n e_reg * out_raw
                        # (tensor_tensor_reduce crashes on this HW/runtime --
                        #  use plain mult + reduce on DVE instead)
                        scr2 = scr_pool.tile([P, SUB], f32, tag="scr")
                        nc.vector.scalar_tensor_tensor(
                            out=scr2[:, 0:w],
                            in0=er[:, 0:w],
                            scalar=1.0,
                            in1=po[h][:, 0:w],
                            op0=mybir.AluOpType.mult,
                            op1=mybir.AluOpType.mult,
                            accum_out=cols[(2, h)][:, si : si + 1],
                        )
                    si += 1

            stat_sb = stat_pool.tile([P, 6], f32)
            for q in range(3):
                for h in range(2):
                    nc.vector.reduce_sum(
                        out=stat_sb[:, q * 2 + h : q * 2 + h + 1],
                        in_=cols[(q, h)][:],
                        axis=mybir.AxisListType.X,
                    )
            nc.sync.dma_start(stats[:], stat_sb[:])

    nc.compile()
    return nc


def _get_program():
    global _PROG
    if _PROG is None:
        _PROG = _build_program()
    return _PROG


def _l2norm_rows(x):
    n = np.sqrt((x.astype(np.float64) ** 2).sum(axis=1, keepdims=True))
    return (x / np.maximum(n, 1e-12)).astype(np.float32)


def kernel(inputs1, inputs2, features, targets):
    global LAST_RESULTS
    inputs1 = np.ascontiguousarray(np.asarray(inputs1, dtype=np.float32))
    inputs2 = np.ascontiguousarray(np.asarray(inputs2, dtype=np.float32))
    features = np.ascontiguousarray(np.asarray(features, dtype=np.float32))
    targets_np = np.asarray(targets)
    tgt = targets_np.astype(np.int64)

    x1 = _l2norm_rows(inputs1)
    x2 = _l2norm_rows(inputs2)
    x1t = np.ascontiguousarray(x1.T)
    x2t = np.ascontiguousarray(x2.T)

    nc = _get_program()
    in_maps = [
        {
            "feat": features[c * NS : (c + 1) * NS],
            "x1t": x1t,
            "x2t": x2t,
        }
        for c in range(N_CORES)
    ]
    res = run_bass_kernel_spmd(
        nc, in_maps, core_ids=list(range(N_CORES)), trace=TRACE
    )
    LAST_RESULTS = res

    # ---- combine per-shard softmax partial sums (host, tiny) ----
    # stats cols: 0,1 = l_out (b 0:128, 128:256); 2,3 = l_reg; 4,5 = t1
    st = np.stack([res.results[c]["stats"] for c in range(N_CORES)], axis=0)
    st = st.astype(np.float64)
    l_out = st[:, :, 0:2].sum(axis=0).T.reshape(B)  # [B]
    l_reg = st[:, :, 2:4].sum(axis=0).T.reshape(B)
    t1 = st[:, :, 4:6].sum(axis=0).T.reshape(B)

    lse_out = SHIFT + np.log(l_out)  # log sum_n exp(o[b,n]), o = 20*dot
    s1 = INV_TEMP * t1 / l_reg  # sum_n softmax(reg)[b,n] * o[b,n]

    x1_64 = x1.astype(np.float64)
    frows = features[tgt].astype(np.float64)
    d = (x1_64 * frows).sum(axis=1)  # [B] sim to own centroid
    o_t = INV_TEMP * d

    loss_c = np.mean(lse_out - o_t)
    loss_s = np.mean(
        (1.0 - SMOOTH) * (lse_out - s1) + SMOOTH * (lse_out - o_t)
    )

    # ---- segment-softmax memory update rows (host, <=256 rows) ----
    logits_w = -d / TAU_W
    uniq, inv = np.unique(tgt, return_inverse=True)
    nu = len(uniq)
    m_u = np.full(nu, -np.inf)
    np.maximum.at(m_u, inv, logits_w)
    e = np.exp(logits_w - m_u[inv])
    den_u = np.zeros(nu)
    np.add.at(den_u, inv, e)
    wgt = e / den_u[inv]
    wmean_u = np.zeros((nu, D))
    np.add.at(wmean_u, inv, wgt[:, None] * x1_64)
    upd = MOMENTUM * features[uniq].astype(np.float64) + (1.0 - MOMENTUM) * wmean_u
    un = np.sqrt((upd**2).sum(axis=1, keepdims=True))
    upd = (upd / np.maximum(un, 1e-12)).astype(np.float32)

    # ---- gather shards + scatter updated rows ----
    new_features = np.concatenate(
        [res.results[c]["newfeat"] for c in range(N_CORES)], axis=0
    )
    new_features[uniq] = upd

    return (
        np.float32(loss_c),
        np.float32(loss_s),
        new_features,
    )
